# revision 16
# baseline (speedup 1.0000x reference)
"""Bidirectional-ALiBi bias kernel for Trainium2 (Bass/Tile), 8-core SPMD.

Computes out[h, i, j] = |j - i| * m where m = alpha[h] on the first
row/column, gamma[h] above the diagonal, beta[h] below it, and 0 on the
(non-edge) diagonal.  Output [16, 2048, 2048] f32, sharded 2 heads/core.

Strategy ("aligned full-row tiles"): every interior row i is a shifted
window of a per-head profile V(k) = gamma*max(k,0) + beta*max(-k,0),
k = j - i.  Each core computes a per-head W image W[p, c] =
V(c - p - (S-1)) for c in [127, 4095), split into a hi chunk
[2047,4095) and lo chunk [127,2047).  Each 128-row output block t
becomes a PRIVATE SBUF tile [128, 2048] built from 1-2 window copies
out of W, then patched in SBUF: column 0 <- alpha*i (from R[p,t] =
alpha*(128t+p)) and, for t=0, row 0 <- alpha*j.  The patched tile goes
out as ONE fully contiguous, 8KB-per-row-aligned 1-MiB DMA.  No 4-byte
scatter writes, no misaligned row fragments (the old kernel lost ~25%
of DMA-engine time to those: 4B packets ran at 0.36 GB/s and k*512-byte
fragments at 14-23 GB/s vs 26.0 GB/s for aligned 8KB descriptors;
this version sustains 417-423 GB/s, the 16-engine wall, for the whole
~80us stream).

Engine layout (all measured): the DMA stream is the wall (16 SDMA
engines x 26.0 GB/s on the 32 MiB of writes).  Only sync (SP) and
scalar (Act) have HWDGE rings: sync carries the 16 h0 block DMAs,
scalar the 16 h1 ones.  Vector (DVE) computes the W chunks and feeds
the sync ring (h0 window copies + patches); the Act engine feeds its
own ring (h1 copies + patches via activation-copy) so each
copy->patch->DMA chain stays on one producer engine -- every extra
cross-engine hop costs ~1-2us of semaphore latency.  Gpsimd runs only
the 3 iotas: its tensor_scalar has a ~13us software launch cost that
must stay off the critical path (putting R/row-patch ops there cost
the first version 38us of ramp).  The t=0 tile's right half ships as a
separate DMA as soon as the copy+row-patch land, ~2us before the
column patch completes the left half.
"""

import numpy as np

H = 16
S = 2048
P = 128
N_CORES = 8
H_LOC = H // N_CORES  # 2 heads per core
NT = S // P  # 16 row blocks per head

C_LO = 127      # lowest W column needed (t=15 window starts at 2047-1920)
HI0 = 2047      # hi chunk covers c in [2047, 4095); lo covers [127, 2047)
W_HI = 2048
W_LO = HI0 - C_LO  # 1920

_NC = None


def _build(nbuf=10, copy_h1_eng="scalar", ring_h1="scalar"):
    import concourse.bacc as bacc
    import concourse.mybir as mybir
    from concourse.tile import TileContext

    f32 = mybir.dt.float32
    nc = bacc.Bacc("TRN2", target_bir_lowering=False, debug=False)

    alpha_d = nc.dram_tensor("alpha", [H_LOC], f32, kind="ExternalInput").ap()
    beta_d = nc.dram_tensor("beta", [H_LOC], f32, kind="ExternalInput").ap()
    gamma_d = nc.dram_tensor("gamma", [H_LOC], f32, kind="ExternalInput").ap()
    out_d = nc.dram_tensor("out", [H_LOC, S, S], f32, kind="ExternalOutput").ap()

    with TileContext(nc) as tc:
        rings = {"h0": nc.sync, "h1": getattr(nc, ring_h1)}
        copy_eng = {0: nc.vector, 1: getattr(nc, copy_h1_eng)}

        with (
            tc.tile_pool(name="coef", bufs=1) as cpool,
            tc.tile_pool(name="kpool", bufs=1) as kpool,
            tc.tile_pool(name="wpool", bufs=1) as wpool,
            tc.tile_pool(name="t2pool", bufs=2) as t2pool,
            tc.tile_pool(name="tpool", bufs=nbuf) as tpool,
        ):
            # per-head coefficients broadcast to all partitions: [128, 2].
            # G2/B2 gate the W compute; A2 only gates the (later) patches.
            G2 = cpool.tile([P, H_LOC], f32)
            nc.sync.dma_start(out=G2[:], in_=gamma_d.partition_broadcast(P))
            B2 = cpool.tile([P, H_LOC], f32)
            rings["h1"].dma_start(out=B2[:], in_=beta_d.partition_broadcast(P))
            A2 = cpool.tile([P, H_LOC], f32)
            nc.sync.dma_start(out=A2[:], in_=alpha_d.partition_broadcast(P))
            NB2 = cpool.tile([P, H_LOC], f32)

            # K iotas: Khi[p, cc] = cc - p  (c = cc + 2047, k = c - p - 2047)
            #          Klo[p, cc] = cc - p - 1920  (c = cc + 127)
            Khi = kpool.tile([P, W_HI], f32, tag="Khi")
            nc.gpsimd.iota(
                Khi[:],
                pattern=[[1, W_HI]],
                base=0,
                channel_multiplier=-1,
                allow_small_or_imprecise_dtypes=True,
            )
            # IB[p, t] = 128t + p, for the column-0 patch sources
            IB = cpool.tile([P, NT], f32, tag="IB")
            nc.gpsimd.iota(
                IB[:],
                pattern=[[P, NT]],
                base=0,
                channel_multiplier=1,
                allow_small_or_imprecise_dtypes=True,
            )
            Klo = kpool.tile([P, W_LO], f32, tag="Klo")
            nc.gpsimd.iota(
                Klo[:],
                pattern=[[1, W_LO]],
                base=-W_LO,
                channel_multiplier=-1,
                allow_small_or_imprecise_dtypes=True,
            )
            Rs = [None, None]

            # W images.  T2 = max(gamma*k, 0); W = max(-beta*k, T2) -- the
            # two branches are never simultaneously positive so max = sum.
            # T2 first: it needs only G2 (first coef to land), NB2 joins
            # behind it, keeping vector busy while B2 is still in flight.
            first = [True]

            def w_chunk(Kc, w, h, Wout):
                T2 = t2pool.tile([P, W_HI], f32, tag="T2")
                nc.vector.tensor_scalar(
                    out=T2[:, :w],
                    in0=Kc[:, :w],
                    scalar1=G2[:, h : h + 1],
                    scalar2=0.0,
                    op0=mybir.AluOpType.mult,
                    op1=mybir.AluOpType.max,
                )
                if first[0]:
                    first[0] = False
                    nc.vector.tensor_scalar_mul(NB2[:], B2[:], -1.0)
                nc.vector.scalar_tensor_tensor(
                    out=Wout[:],
                    in0=Kc[:, :w],
                    scalar=NB2[:, h : h + 1],
                    in1=T2[:, :w],
                    op0=mybir.AluOpType.mult,
                    op1=mybir.AluOpType.max,
                )

            Whi = [wpool.tile([P, W_HI], f32, tag=f"Whi{h}", name=f"Whi{h}") for h in range(H_LOC)]
            Wlo = [wpool.tile([P, W_LO], f32, tag=f"Wlo{h}", name=f"Wlo{h}") for h in range(H_LOC)]

            def cpy(h, out, in_):
                # h0 tiles copy on vector, h1 tiles on the Act engine, so
                # each DMA ring is fed by exactly one compute engine and the
                # copy -> patch -> dma chain never ping-pongs across engines
                # (each cross-engine semaphore hop costs ~1-2us).
                if copy_eng[h] is nc.vector:
                    nc.vector.tensor_copy(out=out, in_=in_)
                else:
                    copy_eng[h].copy(out=out, in_=in_)

            def emit_tile(h, t):
                if Rs[h] is None:
                    # R[h][p, t] = alpha_h * (128t + p): column-0 patch values
                    Rh = cpool.tile([P, NT], f32, tag=f"R{h}", name=f"R{h}")
                    nc.vector.tensor_scalar_mul(Rh[:], IB[:], A2[:, h : h + 1])
                    Rs[h] = Rh
                T = tpool.tile([P, S], f32, tag="T")
                ring = rings[f"h{h}"]
                half = S // 2
                # window c in [o, o+2048), o = 2047 - 128t
                if t == 0:
                    cpy(h, T[:], Whi[h][:])
                    # row-0 patch first (cheap, needs only Khi+A2):
                    # T[0, j] = alpha_h * j; Khi[0, cc] = cc = j
                    if h == 0:
                        nc.vector.tensor_scalar_mul(
                            T[0:1, :], Khi[0:1, :], A2[0:1, h : h + 1]
                        )
                    else:
                        nc.scalar.mul(T[0:1, :], Khi[0:1, :], A2[0:1, h : h + 1])
                    # the right half is now fully patched: ship it early
                    ring.dma_start(out=out_d[h, 0:P, half:S], in_=T[:, half:S])
                    # column-0 patch only gates the left half
                    cpy(h, T[:, 0:1], Rs[h][:, t : t + 1])
                    ring.dma_start(out=out_d[h, 0:P, 0:half], in_=T[:, 0:half])
                else:
                    jl = P * t  # low piece covers j in [0, 128t)
                    cpy(h, T[:, 0:jl], Wlo[h][:, W_LO - jl : W_LO])
                    cpy(h, T[:, jl:S], Whi[h][:, 0 : S - jl])
                    # column-0 patch: T[p, 0] = alpha_h * (128t + p)
                    cpy(h, T[:, 0:1], Rs[h][:, t : t + 1])
                    ring.dma_start(
                        out=out_d[h, P * t : P * (t + 1), :], in_=T[:]
                    )

            # hi chunks first (they alone serve t=0); interleave the lo
            # chunks between early tile emissions so the DMA stream never
            # starves while vector computes them.
            w_chunk(Khi, W_HI, 0, Whi[0])
            emit_tile(0, 0)
            w_chunk(Khi, W_HI, 1, Whi[1])
            emit_tile(1, 0)
            w_chunk(Klo, W_LO, 0, Wlo[0])
            emit_tile(0, 1)
            w_chunk(Klo, W_LO, 1, Wlo[1])
            emit_tile(1, 1)
            for t in range(2, NT):
                for h in range(H_LOC):
                    emit_tile(h, t)

    nc.compile()
    return nc


def _run(alpha, beta, gamma, **spmd_kwargs):
    """Compile (cached) and run on the 8 NeuronCores; returns BassKernelResults."""
    global _NC
    if _NC is None:
        _NC = _build()
    from concourse import bass_utils

    alpha = np.ascontiguousarray(alpha, dtype=np.float32)
    beta = np.ascontiguousarray(beta, dtype=np.float32)
    gamma = np.ascontiguousarray(gamma, dtype=np.float32)
    in_maps = [
        {
            "alpha": alpha[c * H_LOC : (c + 1) * H_LOC],
            "beta": beta[c * H_LOC : (c + 1) * H_LOC],
            "gamma": gamma[c * H_LOC : (c + 1) * H_LOC],
        }
        for c in range(N_CORES)
    ]
    return bass_utils.run_bass_kernel_spmd(
        _NC, in_maps, core_ids=list(range(N_CORES)), **spmd_kwargs
    )


def kernel(alpha, beta, gamma, seq_len):
    assert int(seq_len) == S, f"kernel hardcodes seq_len={S}, got {seq_len}"
    res = _run(alpha, beta, gamma)
    return np.concatenate([r["out"] for r in res.results], axis=0)


# revision 18
# speedup vs baseline: 1.1182x; 1.1182x over previous
"""Bidirectional-ALiBi bias kernel for Trainium2 (Bass/Tile), 8-core SPMD.

Computes out[h, i, j] = |j - i| * m where m = alpha[h] on the first
row/column, gamma[h] above the diagonal, beta[h] below it, and 0 on the
(non-edge) diagonal.  Output [16, 2048, 2048] f32, sharded 2 heads/core.

Strategy ("aligned full-row tiles"): every interior row i is a shifted
window of a per-head profile V(k) = gamma*max(k,0) + beta*max(-k,0),
k = j - i.  Each core computes a per-head W image W[p, c] =
V(c - p - (S-1)) for c in [127, 4095), split into a hi chunk
[2047,4095) and lo chunk [127,2047).  Each 128-row output block t
becomes a PRIVATE SBUF tile [128, 2048] built from 1-2 window copies
out of W, then patched in SBUF: column 0 <- alpha*i (from R[p,t] =
alpha*(128t+p)) and, for t=0, row 0 <- alpha*j.  The patched tile goes
out as ONE fully contiguous, 8KB-per-row-aligned 1-MiB DMA.  No 4-byte
scatter writes, no misaligned row fragments (the old kernel lost ~25%
of DMA-engine time to those: 4B packets ran at 0.36 GB/s and k*512-byte
fragments at 14-23 GB/s vs 26.0 GB/s for aligned 8KB descriptors;
this version sustains 417-423 GB/s, the 16-engine wall, for the whole
~80us stream).

Engine layout (all measured): the DMA stream is the wall (16 SDMA
engines x 26.0 GB/s on the 32 MiB of writes).  Only sync (SP) and
scalar (Act) have HWDGE rings: sync carries the 16 h0 block DMAs,
scalar the 16 h1 ones.  Vector (DVE) computes the W chunks and feeds
the sync ring (h0 window copies + patches); the Act engine feeds its
own ring (h1 copies + patches via activation-copy) so each
copy->patch->DMA chain stays on one producer engine -- every extra
cross-engine hop costs ~1-2us of semaphore latency.  Gpsimd runs only
the 3 iotas: its tensor_scalar has a ~13us software launch cost that
must stay off the critical path (putting R/row-patch ops there cost
the first version 38us of ramp).  The t=0 tile's right half ships as a
separate DMA as soon as the copy+row-patch land, ~2us before the
column patch completes the left half.
"""

import numpy as np

H = 16
S = 2048
P = 128
N_CORES = 8
H_LOC = H // N_CORES  # 2 heads per core
NT = S // P  # 16 row blocks per head

C_LO = 127      # lowest W column needed (t=15 window starts at 2047-1920)
HI0 = 2047      # hi chunk covers c in [2047, 4095); lo covers [127, 2047)
W_HI = 2048
W_LO = HI0 - C_LO  # 1920

_NC = None


def _build(nbuf=10, copy_h1_eng="scalar", ring_h1="scalar"):
    import concourse.bacc as bacc
    import concourse.mybir as mybir
    from concourse.tile import TileContext

    f32 = mybir.dt.float32
    nc = bacc.Bacc("TRN2", target_bir_lowering=False, debug=False)

    alpha_d = nc.dram_tensor("alpha", [H_LOC], f32, kind="ExternalInput").ap()
    beta_d = nc.dram_tensor("beta", [H_LOC], f32, kind="ExternalInput").ap()
    gamma_d = nc.dram_tensor("gamma", [H_LOC], f32, kind="ExternalInput").ap()
    out_d = nc.dram_tensor("out", [H_LOC, S, S], f32, kind="ExternalOutput").ap()

    with TileContext(nc) as tc:
        rings = {"h0": nc.sync, "h1": getattr(nc, ring_h1)}
        copy_eng = {0: nc.vector, 1: getattr(nc, copy_h1_eng)}

        with (
            tc.tile_pool(name="coef", bufs=1) as cpool,
            tc.tile_pool(name="kpool", bufs=1) as kpool,
            tc.tile_pool(name="wpool", bufs=1) as wpool,
            tc.tile_pool(name="t2pool", bufs=2) as t2pool,
            tc.tile_pool(name="tpool", bufs=nbuf) as tpool,
        ):
            # per-head coefficients broadcast to all partitions: [128, 2].
            # G2/B2 gate the W compute; A2 only gates the (later) patches.
            G2 = cpool.tile([P, H_LOC], f32)
            nc.sync.dma_start(out=G2[:], in_=gamma_d.partition_broadcast(P))
            B2 = cpool.tile([P, H_LOC], f32)
            rings["h1"].dma_start(out=B2[:], in_=beta_d.partition_broadcast(P))
            A2 = cpool.tile([P, H_LOC], f32)
            nc.sync.dma_start(out=A2[:], in_=alpha_d.partition_broadcast(P))
            # NB2 emitted FIRST on vector: the greedy Tile scheduler picks
            # the highest-priority READY op whenever an engine frees up, so
            # every link of the first tile's chain must be dep-complete the
            # moment its predecessor retires or unrelated work cuts in.
            # With NB2 done up front, Whi0 is ready the instant T2hi0 ends.
            NB2 = cpool.tile([P, H_LOC], f32)
            nc.vector.tensor_scalar_mul(NB2[:], B2[:], -1.0)

            # K iotas: Khi[p, cc] = cc - p  (c = cc + 2047, k = c - p - 2047)
            #          Klo[p, cc] = cc - p - 1920  (c = cc + 127)
            Khi = kpool.tile([P, W_HI], f32, tag="Khi")
            nc.gpsimd.iota(
                Khi[:],
                pattern=[[1, W_HI]],
                base=0,
                channel_multiplier=-1,
                allow_small_or_imprecise_dtypes=True,
            )
            # IB[p, t] = 128t + p, for the column-0 patch sources
            IB = cpool.tile([P, NT], f32, tag="IB")
            nc.gpsimd.iota(
                IB[:],
                pattern=[[P, NT]],
                base=0,
                channel_multiplier=1,
                allow_small_or_imprecise_dtypes=True,
            )
            Klo = kpool.tile([P, W_LO], f32, tag="Klo")
            nc.gpsimd.iota(
                Klo[:],
                pattern=[[1, W_LO]],
                base=-W_LO,
                channel_multiplier=-1,
                allow_small_or_imprecise_dtypes=True,
            )
            Rs = [None, None]

            # W images.  T2 = max(gamma*k, 0); W = max(-beta*k, T2) -- the
            # two branches are never simultaneously positive so max = sum.
            # h1's T2 runs as Relu(k*gamma) on the Act engine (idle early,
            # in parallel with vector's h0 chain); the STT stays on vector.
            def w_chunk(Kc, w, h, Wout):
                T2 = t2pool.tile([P, W_HI], f32, tag="T2")
                if h == 1:
                    nc.scalar.activation(
                        out=T2[:, :w],
                        in_=Kc[:, :w],
                        func=mybir.ActivationFunctionType.Relu,
                        scale=G2[:, h : h + 1],
                    )
                else:
                    nc.vector.tensor_scalar(
                        out=T2[:, :w],
                        in0=Kc[:, :w],
                        scalar1=G2[:, h : h + 1],
                        scalar2=0.0,
                        op0=mybir.AluOpType.mult,
                        op1=mybir.AluOpType.max,
                    )
                nc.vector.scalar_tensor_tensor(
                    out=Wout[:],
                    in0=Kc[:, :w],
                    scalar=NB2[:, h : h + 1],
                    in1=T2[:, :w],
                    op0=mybir.AluOpType.mult,
                    op1=mybir.AluOpType.max,
                )

            Whi = [wpool.tile([P, W_HI], f32, tag=f"Whi{h}", name=f"Whi{h}") for h in range(H_LOC)]
            Wlo = [wpool.tile([P, W_LO], f32, tag=f"Wlo{h}", name=f"Wlo{h}") for h in range(H_LOC)]

            def cpy(h, out, in_):
                # h0 tiles copy on vector, h1 tiles on the Act engine, so
                # each DMA ring is fed by exactly one compute engine and the
                # copy -> patch -> dma chain never ping-pongs across engines
                # (each cross-engine semaphore hop costs ~1-2us).
                if copy_eng[h] is nc.vector:
                    nc.vector.tensor_copy(out=out, in_=in_)
                else:
                    copy_eng[h].copy(out=out, in_=in_)

            def emit_tile(h, t):
                if Rs[h] is None:
                    # R[h][p, t] = alpha_h * (128t + p): column-0 patch values
                    Rh = cpool.tile([P, NT], f32, tag=f"R{h}", name=f"R{h}")
                    nc.vector.tensor_scalar_mul(Rh[:], IB[:], A2[:, h : h + 1])
                    Rs[h] = Rh
                T = tpool.tile([P, S], f32, tag="T")
                ring = rings[f"h{h}"]
                half = S // 2
                # window c in [o, o+2048), o = 2047 - 128t
                if t == 0:
                    cpy(h, T[:], Whi[h][:])
                    # row-0 patch first (cheap, needs only Khi+A2):
                    # T[0, j] = alpha_h * j; Khi[0, cc] = cc = j
                    if h == 0:
                        nc.vector.tensor_scalar_mul(
                            T[0:1, :], Khi[0:1, :], A2[0:1, h : h + 1]
                        )
                    else:
                        nc.scalar.mul(T[0:1, :], Khi[0:1, :], A2[0:1, h : h + 1])
                    # the right half is now fully patched: ship it early
                    ring.dma_start(out=out_d[h, 0:P, half:S], in_=T[:, half:S])
                    # column-0 patch only gates the left half
                    cpy(h, T[:, 0:1], Rs[h][:, t : t + 1])
                    ring.dma_start(out=out_d[h, 0:P, 0:half], in_=T[:, 0:half])
                else:
                    jl = P * t  # low piece covers j in [0, 128t)
                    cpy(h, T[:, 0:jl], Wlo[h][:, W_LO - jl : W_LO])
                    cpy(h, T[:, jl:S], Whi[h][:, 0 : S - jl])
                    # column-0 patch: T[p, 0] = alpha_h * (128t + p)
                    cpy(h, T[:, 0:1], Rs[h][:, t : t + 1])
                    ring.dma_start(
                        out=out_d[h, P * t : P * (t + 1), :], in_=T[:]
                    )

            # hi chunks first (they alone serve t=0); interleave the lo
            # chunks between early tile emissions so the DMA stream never
            # starves while vector computes them.
            w_chunk(Khi, W_HI, 0, Whi[0])
            emit_tile(0, 0)
            w_chunk(Khi, W_HI, 1, Whi[1])
            emit_tile(1, 0)
            w_chunk(Klo, W_LO, 0, Wlo[0])
            emit_tile(0, 1)
            w_chunk(Klo, W_LO, 1, Wlo[1])
            emit_tile(1, 1)
            for t in range(2, NT):
                for h in range(H_LOC):
                    emit_tile(h, t)

    nc.compile()
    return nc


def _run(alpha, beta, gamma, **spmd_kwargs):
    """Compile (cached) and run on the 8 NeuronCores; returns BassKernelResults."""
    global _NC
    if _NC is None:
        _NC = _build()
    from concourse import bass_utils

    alpha = np.ascontiguousarray(alpha, dtype=np.float32)
    beta = np.ascontiguousarray(beta, dtype=np.float32)
    gamma = np.ascontiguousarray(gamma, dtype=np.float32)
    in_maps = [
        {
            "alpha": alpha[c * H_LOC : (c + 1) * H_LOC],
            "beta": beta[c * H_LOC : (c + 1) * H_LOC],
            "gamma": gamma[c * H_LOC : (c + 1) * H_LOC],
        }
        for c in range(N_CORES)
    ]
    return bass_utils.run_bass_kernel_spmd(
        _NC, in_maps, core_ids=list(range(N_CORES)), **spmd_kwargs
    )


def kernel(alpha, beta, gamma, seq_len):
    assert int(seq_len) == S, f"kernel hardcodes seq_len={S}, got {seq_len}"
    res = _run(alpha, beta, gamma)
    return np.concatenate([r["out"] for r in res.results], axis=0)
